# revision 1
# baseline (speedup 1.0000x reference)
"""Trainium2 Bass kernel for nn_MixtureOfExperts (dense MoE, E=8 experts).

Data-parallel over 8 NeuronCores: each core processes B/8 = 2048 tokens with
all expert + gate weights replicated. No collectives.

Per-core algorithm (feature-major intermediates, fp32r matmuls):
  xT = concat(text, tech).T                     # [IN_D, T] host-side marshalling
  gates = softmax(x @ Wg + bg)                  # token-major [T, E]
  per expert e:
    hT_e = relu(W1[e].T @ xT + b1[e])           # feature-major [OUT_D, T]
    y_e  = hT_e.T @ W2[e]                       # token-major tiles, PSUM
    out += gates[:, e] * y_e                    # fused DVE scalar_tensor_tensor
  out += gates @ b2  (bias matmul, K=E)
"""
import os
import numpy as np
from contextlib import ExitStack

import concourse.bass as bass
import concourse.mybir as mybir
import concourse.tile as tile
from concourse import bacc
from concourse.bass_utils import run_bass_kernel_spmd

B, TEXT_D, TECH_D = 16384, 768, 256
IN_D, OUT_D, E = 1024, 1024, 8
NCORES = 8
T = B // NCORES          # 2048 tokens per core
C = 1024                 # token chunk per outer pass
NCH = T // C             # 2 chunks
P = 128
XT_BUFS = 1
KT = IN_D // P           # 8 k-tiles (also OUT_D // P)
FT = OUT_D // P          # 8 feature tiles
TT = C // P              # 8 token tiles per chunk
NC2 = C // 512           # 2 half-chunks of 512 tokens
F32 = mybir.dt.float32
F32R = mybir.dt.float32r
AF = mybir.ActivationFunctionType
ALU = mybir.AluOpType
AX = mybir.AxisListType


def build_kernel(nc: bass.Bass, reps: int = 1):
    from concourse.masks import make_identity

    xt_in = nc.dram_tensor("xt_in", [IN_D, T], F32R, kind="ExternalInput")
    W1 = nc.dram_tensor("W1", [E, IN_D, OUT_D], F32R, kind="ExternalInput")
    b1 = nc.dram_tensor("b1", [E, OUT_D], F32, kind="ExternalInput")
    W2 = nc.dram_tensor("W2", [E, OUT_D, OUT_D], F32R, kind="ExternalInput")
    b2 = nc.dram_tensor("b2", [E, OUT_D], F32R, kind="ExternalInput")
    Wg = nc.dram_tensor("Wg", [IN_D, E], F32R, kind="ExternalInput")
    bg = nc.dram_tensor("bg", [1, E], F32R, kind="ExternalInput")
    ones_in = nc.dram_tensor("ones_in", [1, P], F32R, kind="ExternalInput")
    out = nc.dram_tensor("out", [T, OUT_D], F32, kind="ExternalOutput")

    with tile.TileContext(nc) as tc, ExitStack() as ctx:
        consts = ctx.enter_context(tc.tile_pool(name="consts", bufs=1))
        xt_p = ctx.enter_context(tc.tile_pool(name="xt", bufs=XT_BUFS))
        h_p = ctx.enter_context(tc.tile_pool(name="h", bufs=1))
        oacc_p = ctx.enter_context(tc.tile_pool(name="oacc", bufs=1))
        w1_p = ctx.enter_context(tc.tile_pool(name="w1", bufs=4))
        w2_p = ctx.enter_context(tc.tile_pool(name="w2", bufs=2))
        gates_p = ctx.enter_context(tc.tile_pool(name="gates", bufs=1))
        gt_p = ctx.enter_context(tc.tile_pool(name="gt", bufs=2))
        sm_p = ctx.enter_context(tc.tile_pool(name="sm", bufs=2))
        # PSUM pools: xp 2 + lg 1 + ph 2 + y 3 = 8 banks
        pp_xp = ctx.enter_context(tc.tile_pool(name="pp_xp", bufs=1, space="PSUM"))
        pp_lg = ctx.enter_context(tc.tile_pool(name="pp_lg", bufs=1, space="PSUM"))
        pp_h = ctx.enter_context(tc.tile_pool(name="pp_h", bufs=3, space="PSUM"))
        pp_y = ctx.enter_context(tc.tile_pool(name="pp_y", bufs=3, space="PSUM"))

        identity = consts.tile([P, P], F32)
        make_identity(nc, identity[:])
        ones_r = consts.tile([1, P], F32R)
        nc.sync.dma_start(ones_r[:], ones_in[:, :])
        b1sb = consts.tile([P, E, FT], F32)
        nc.sync.dma_start(b1sb[:], b1.rearrange("e (ft p) -> p e ft", p=P))
        b2sb = consts.tile([E, OUT_D], F32R)
        nc.sync.dma_start(b2sb[:], b2[:, :])
        wg_sb = consts.tile([P, KT, E], F32R)
        nc.sync.dma_start(wg_sb[:], Wg.rearrange("(ko p) e -> p ko e", p=P))
        bg_sb = consts.tile([1, E], F32R)
        nc.sync.dma_start(bg_sb[:], bg[:, :])

        for ch in range(NCH * reps):
            t0 = (ch % NCH) * C
            xt = xt_p.tile([P, KT, C], F32R, tag="xt")
            gates = gates_p.tile([P, TT, E], F32, tag="gates")
            oacc = oacc_p.tile([P, TT, OUT_D], F32, tag="oacc")

            # ---- load feature-major xT chunk (host pre-transposed) ----
            for k in range(KT):
                nc.sync.dma_start(xt[:, k], xt_in[k * P:(k + 1) * P, t0:t0 + C])

            # ---- gating: logits -> softmax -> gates; transpose -> bias init ----
            for t in range(TT):
                lg = pp_lg.tile([P, E], F32, tag="lg")
                nc.tensor.matmul(lg[:], ones_r[:], bg_sb[:], start=True, stop=False)
                for k in range(KT):
                    nc.tensor.matmul(lg[:], xt[:, k, t * P:(t + 1) * P], wg_sb[:, k],
                                     start=False, stop=(k == KT - 1))
                nmx = sm_p.tile([P, 1], F32, tag="nmx")
                nc.vector.reduce_max(nmx[:], lg[:], axis=AX.X)
                nc.vector.tensor_scalar_mul(nmx[:], nmx[:], -1.0)
                g_t = gates[:, t]
                nc.scalar.activation(g_t, lg[:], AF.Exp, bias=nmx[:])
                sm = sm_p.tile([P, 1], F32, tag="sm")
                nc.vector.reduce_sum(sm[:], g_t, axis=AX.X)
                nc.vector.reciprocal(sm[:], sm[:])
                nc.vector.tensor_scalar_mul(g_t, g_t, sm[:])
                # transpose gates tile -> gT [E, P], then bias matmul initializes oacc
                gtp = pp_xp.tile([P, P], F32, tag="xp")
                nc.tensor.transpose(gtp[:E, :], g_t, identity[:])
                gt = gt_p.tile([E, P], F32R, tag="gt")
                nc.vector.tensor_copy(gt[:], gtp[:E, :])
                for c2 in range(2):
                    yb = pp_y.tile([P, 512], F32, tag="y")
                    nc.tensor.matmul(yb[:], gt[:], b2sb[:, c2 * 512:(c2 + 1) * 512],
                                     start=True, stop=True)
                    nc.vector.tensor_copy(oacc[:, t, c2 * 512:(c2 + 1) * 512], yb[:])

            # ---- expert loop ----
            for e in range(E):
                w2sb = w2_p.tile([P, KT, OUT_D], F32R, tag="w2")
                for ko in range(KT):
                    nc.sync.dma_start(w2sb[:, ko], W2[e, ko * P:(ko + 1) * P, :])
                h = h_p.tile([P, FT, C], F32R, tag="h")
                # layer 1: hT[f, :] = relu(W1[e][:, f].T @ xT + b1[e][f])
                for f in range(FT):
                    w1g = w1_p.tile([P, KT, P], F32R, tag="w1")
                    nc.sync.dma_start(
                        w1g[:], W1[e, :, f * P:(f + 1) * P].rearrange("(ko p) f -> p ko f", p=P))
                    for c2 in range(NC2):
                        ph = pp_h.tile([P, 512], F32, tag="ph")
                        cs = bass.ds(c2 * 512, 512)
                        for k in range(KT):
                            nc.tensor.matmul(ph[:], w1g[:, k], xt[:, k, cs],
                                             start=(k == 0), stop=(k == KT - 1))
                        nc.scalar.activation(h[:, f, cs], ph[:], AF.Relu,
                                             bias=b1sb[:, e, f:f + 1])
                # layer 2 + gated accumulate
                for t in range(TT):
                    y0 = pp_y.tile([P, 512], F32, tag="y")
                    y1 = pp_y.tile([P, 512], F32, tag="y")
                    for k in range(KT):
                        hk = h[:, k, t * P:(t + 1) * P]
                        nc.tensor.matmul(y0[:], hk, w2sb[:, k, 0:512],
                                         start=(k == 0), stop=(k == KT - 1))
                        nc.tensor.matmul(y1[:], hk, w2sb[:, k, 512:1024],
                                         start=(k == 0), stop=(k == KT - 1))
                    g_e = gates[:, t, e:e + 1]
                    nc.vector.scalar_tensor_tensor(
                        oacc[:, t, 0:512], y0[:], g_e, oacc[:, t, 0:512],
                        op0=ALU.mult, op1=ALU.add)
                    nc.vector.scalar_tensor_tensor(
                        oacc[:, t, 512:1024], y1[:], g_e, oacc[:, t, 512:1024],
                        op0=ALU.mult, op1=ALU.add)

            # ---- write out chunk ----
            for t in range(TT):
                nc.sync.dma_start(out[t0 + t * P:t0 + (t + 1) * P, :], oacc[:, t])
    return nc


_ONES = np.ones((1, P), dtype=np.float32)

_compiled = {}


def _get_compiled(reps: int = 1):
    if reps not in _compiled:
        nc = bacc.Bacc(None, target_bir_lowering=False)
        build_kernel(nc, reps)
        nc.finalize()
        _compiled[reps] = nc
    return _compiled[reps]


LAST_RESULTS = None


def kernel(text_features, technical_features, W1, b1, W2, b2, Wg, bg):
    global LAST_RESULTS
    nc = _get_compiled()
    text_features = np.asarray(text_features, dtype=np.float32)
    technical_features = np.asarray(technical_features, dtype=np.float32)
    xt_full = np.ascontiguousarray(
        np.concatenate([text_features, technical_features], axis=1).T)
    W1 = np.ascontiguousarray(np.asarray(W1, dtype=np.float32))
    b1 = np.ascontiguousarray(np.asarray(b1, dtype=np.float32))
    W2 = np.ascontiguousarray(np.asarray(W2, dtype=np.float32))
    b2 = np.ascontiguousarray(np.asarray(b2, dtype=np.float32))
    Wg = np.ascontiguousarray(np.asarray(Wg, dtype=np.float32))
    bg = np.ascontiguousarray(np.asarray(bg, dtype=np.float32).reshape(1, E))

    in_maps = []
    for i in range(NCORES):
        sl = slice(i * T, (i + 1) * T)
        in_maps.append({
            "xt_in": np.ascontiguousarray(xt_full[:, sl]),
            "W1": W1, "b1": b1, "W2": W2, "b2": b2, "Wg": Wg, "bg": bg,
            "ones_in": _ONES,
        })
    last_exc = None
    for attempt in range(3):
        try:
            LAST_RESULTS = run_bass_kernel_spmd(nc, in_maps, core_ids=list(range(NCORES)))
            break
        except Exception as e:  # transient device/transfer errors: retry
            last_exc = e
            import time
            time.sleep(2.0 * (attempt + 1))
    else:
        raise last_exc
    return np.concatenate(
        [LAST_RESULTS.results[i]["out"] for i in range(NCORES)], axis=0)



# revision 2
# speedup vs baseline: 413.7296x; 413.7296x over previous
"""Trainium2 Bass kernel for nn_MixtureOfExperts (dense MoE, E=8 experts).

Data-parallel over 8 NeuronCores: each core processes B/8 = 2048 tokens with
all expert + gate weights replicated. No collectives.

Per-core algorithm (feature-major intermediates, fp32r matmuls):
  xT = concat(text, tech).T                     # [IN_D, T] host-side marshalling
  gates = softmax(x @ Wg + bg)                  # token-major [T, E]
  per expert e:
    hT_e = relu(W1[e].T @ xT + b1[e])           # feature-major [OUT_D, T]
    y_e  = hT_e.T @ W2[e]                       # token-major tiles, PSUM
    out += gates[:, e] * y_e                    # fused DVE scalar_tensor_tensor
  out += gates @ b2  (bias matmul, K=E)
"""
import os
import numpy as np
from contextlib import ExitStack

import concourse.bass as bass
import concourse.mybir as mybir
import concourse.tile as tile
from concourse import bacc
from concourse.bass_utils import run_bass_kernel_spmd

B, TEXT_D, TECH_D = 16384, 768, 256
IN_D, OUT_D, E = 1024, 1024, 8
NCORES = 8
T = B // NCORES          # 2048 tokens per core
C = 1024                 # token chunk per outer pass
NCH = T // C             # 2 chunks
P = 128
XT_BUFS = 1
KT = IN_D // P           # 8 k-tiles (also OUT_D // P)
FT = OUT_D // P          # 8 feature tiles
TT = C // P              # 8 token tiles per chunk
NC2 = C // 512           # 2 half-chunks of 512 tokens
F32 = mybir.dt.float32
F32R = mybir.dt.float32r
AF = mybir.ActivationFunctionType
ALU = mybir.AluOpType
AX = mybir.AxisListType


def build_kernel(nc: bass.Bass, reps: int = 1):
    from concourse.masks import make_identity

    xt_in = nc.dram_tensor("xt_in", [IN_D, T], F32R, kind="ExternalInput")
    W1 = nc.dram_tensor("W1", [E, IN_D, OUT_D], F32R, kind="ExternalInput")
    b1 = nc.dram_tensor("b1", [E, OUT_D], F32, kind="ExternalInput")
    W2 = nc.dram_tensor("W2", [E, OUT_D, OUT_D], F32R, kind="ExternalInput")
    b2 = nc.dram_tensor("b2", [E, OUT_D], F32R, kind="ExternalInput")
    Wg = nc.dram_tensor("Wg", [IN_D, E], F32R, kind="ExternalInput")
    bg = nc.dram_tensor("bg", [1, E], F32R, kind="ExternalInput")
    ones_in = nc.dram_tensor("ones_in", [1, P], F32R, kind="ExternalInput")
    out = nc.dram_tensor("out", [T, OUT_D], F32, kind="ExternalOutput")

    with tile.TileContext(nc) as tc, ExitStack() as ctx:
        consts = ctx.enter_context(tc.tile_pool(name="consts", bufs=1))
        xt_p = ctx.enter_context(tc.tile_pool(name="xt", bufs=XT_BUFS))
        h_p = ctx.enter_context(tc.tile_pool(name="h", bufs=1))
        oacc_p = ctx.enter_context(tc.tile_pool(name="oacc", bufs=1))
        w1_p = ctx.enter_context(tc.tile_pool(name="w1", bufs=4))
        w2_p = ctx.enter_context(tc.tile_pool(name="w2", bufs=2))
        gates_p = ctx.enter_context(tc.tile_pool(name="gates", bufs=1))
        gt_p = ctx.enter_context(tc.tile_pool(name="gt", bufs=2))
        sm_p = ctx.enter_context(tc.tile_pool(name="sm", bufs=2))
        # PSUM pools: xp 2 + lg 1 + ph 2 + y 3 = 8 banks
        pp_xp = ctx.enter_context(tc.tile_pool(name="pp_xp", bufs=1, space="PSUM"))
        pp_lg = ctx.enter_context(tc.tile_pool(name="pp_lg", bufs=1, space="PSUM"))
        pp_h = ctx.enter_context(tc.tile_pool(name="pp_h", bufs=3, space="PSUM"))
        pp_y = ctx.enter_context(tc.tile_pool(name="pp_y", bufs=3, space="PSUM"))

        identity = consts.tile([P, P], F32)
        make_identity(nc, identity[:])
        ones_r = consts.tile([1, P], F32R)
        nc.sync.dma_start(ones_r[:], ones_in[:, :])
        b1sb = consts.tile([P, E, FT], F32)
        nc.sync.dma_start(b1sb[:], b1.rearrange("e (ft p) -> p e ft", p=P))
        b2sb = consts.tile([E, OUT_D], F32R)
        nc.sync.dma_start(b2sb[:], b2[:, :])
        wg_sb = consts.tile([P, KT, E], F32R)
        nc.sync.dma_start(wg_sb[:], Wg.rearrange("(ko p) e -> p ko e", p=P))
        bg_sb = consts.tile([1, E], F32R)
        nc.sync.dma_start(bg_sb[:], bg[:, :])

        for ch in range(NCH * reps):
            t0 = (ch % NCH) * C
            xt = xt_p.tile([P, KT, C], F32R, tag="xt")
            gates = gates_p.tile([P, TT, E], F32, tag="gates")
            oacc = oacc_p.tile([P, TT, OUT_D], F32, tag="oacc")

            # ---- load feature-major xT chunk (host pre-transposed) ----
            for k in range(KT):
                nc.sync.dma_start(xt[:, k], xt_in[k * P:(k + 1) * P, t0:t0 + C])

            # ---- gating: logits -> softmax -> gates; transpose -> bias init ----
            for t in range(TT):
                lg = pp_lg.tile([P, E], F32, tag="lg")
                nc.tensor.matmul(lg[:], ones_r[:], bg_sb[:], start=True, stop=False)
                for k in range(KT):
                    nc.tensor.matmul(lg[:], xt[:, k, t * P:(t + 1) * P], wg_sb[:, k],
                                     start=False, stop=(k == KT - 1))
                nmx = sm_p.tile([P, 1], F32, tag="nmx")
                nc.vector.reduce_max(nmx[:], lg[:], axis=AX.X)
                nc.vector.tensor_scalar_mul(nmx[:], nmx[:], -1.0)
                g_t = gates[:, t]
                nc.scalar.activation(g_t, lg[:], AF.Exp, bias=nmx[:])
                sm = sm_p.tile([P, 1], F32, tag="sm")
                nc.vector.reduce_sum(sm[:], g_t, axis=AX.X)
                nc.vector.reciprocal(sm[:], sm[:])
                nc.vector.tensor_scalar_mul(g_t, g_t, sm[:])
                # transpose gates tile -> gT [E, P], then bias matmul initializes oacc
                gtp = pp_xp.tile([P, P], F32, tag="xp")
                nc.tensor.transpose(gtp[:E, :], g_t, identity[:])
                gt = gt_p.tile([E, P], F32R, tag="gt")
                nc.vector.tensor_copy(gt[:], gtp[:E, :])
                for c2 in range(2):
                    yb = pp_y.tile([P, 512], F32, tag="y")
                    nc.tensor.matmul(yb[:], gt[:], b2sb[:, c2 * 512:(c2 + 1) * 512],
                                     start=True, stop=True)
                    nc.vector.tensor_copy(oacc[:, t, c2 * 512:(c2 + 1) * 512], yb[:])

            # ---- expert loop ----
            for e in range(E):
                w2sb = w2_p.tile([P, KT, OUT_D], F32R, tag="w2")
                for ko in range(KT):
                    nc.sync.dma_start(w2sb[:, ko], W2[e, ko * P:(ko + 1) * P, :])
                h = h_p.tile([P, FT, C], F32R, tag="h")
                # layer 1: hT[f, :] = relu(W1[e][:, f].T @ xT + b1[e][f])
                for f in range(FT):
                    w1g = w1_p.tile([P, KT, P], F32R, tag="w1")
                    nc.sync.dma_start(
                        w1g[:], W1[e, :, f * P:(f + 1) * P].rearrange("(ko p) f -> p ko f", p=P))
                    for c2 in range(NC2):
                        ph = pp_h.tile([P, 512], F32, tag="ph")
                        cs = bass.ds(c2 * 512, 512)
                        for k in range(KT):
                            nc.tensor.matmul(ph[:], w1g[:, k], xt[:, k, cs],
                                             start=(k == 0), stop=(k == KT - 1))
                        nc.scalar.activation(h[:, f, cs], ph[:], AF.Relu,
                                             bias=b1sb[:, e, f:f + 1])
                # layer 2 + gated accumulate
                for t in range(TT):
                    y0 = pp_y.tile([P, 512], F32, tag="y")
                    y1 = pp_y.tile([P, 512], F32, tag="y")
                    for k in range(KT):
                        hk = h[:, k, t * P:(t + 1) * P]
                        nc.tensor.matmul(y0[:], hk, w2sb[:, k, 0:512],
                                         start=(k == 0), stop=(k == KT - 1))
                        nc.tensor.matmul(y1[:], hk, w2sb[:, k, 512:1024],
                                         start=(k == 0), stop=(k == KT - 1))
                    g_e = gates[:, t, e:e + 1]
                    nc.vector.scalar_tensor_tensor(
                        oacc[:, t, 0:512], y0[:], g_e, oacc[:, t, 0:512],
                        op0=ALU.mult, op1=ALU.add)
                    nc.vector.scalar_tensor_tensor(
                        oacc[:, t, 512:1024], y1[:], g_e, oacc[:, t, 512:1024],
                        op0=ALU.mult, op1=ALU.add)

            # ---- write out chunk ----
            for t in range(TT):
                nc.sync.dma_start(out[t0 + t * P:t0 + (t + 1) * P, :], oacc[:, t])
    return nc


_ONES = np.ones((1, P), dtype=np.float32)

_compiled = {}


def _get_compiled(reps: int = 1):
    if reps not in _compiled:
        nc = bacc.Bacc(None, target_bir_lowering=False)
        build_kernel(nc, reps)
        nc.finalize()
        _compiled[reps] = nc
    return _compiled[reps]


LAST_RESULTS = None


def make_in_maps(inputs):
    text_features = np.asarray(inputs["text_features"], dtype=np.float32)
    technical_features = np.asarray(inputs["technical_features"], dtype=np.float32)
    xt_full = np.ascontiguousarray(
        np.concatenate([text_features, technical_features], axis=1).T)
    W1 = np.ascontiguousarray(np.asarray(inputs["W1"], dtype=np.float32))
    b1 = np.ascontiguousarray(np.asarray(inputs["b1"], dtype=np.float32))
    W2 = np.ascontiguousarray(np.asarray(inputs["W2"], dtype=np.float32))
    b2 = np.ascontiguousarray(np.asarray(inputs["b2"], dtype=np.float32))
    Wg = np.ascontiguousarray(np.asarray(inputs["Wg"], dtype=np.float32))
    bg = np.ascontiguousarray(
        np.asarray(inputs["bg"], dtype=np.float32).reshape(1, E))

    in_maps = []
    for i in range(NCORES):
        sl = slice(i * T, (i + 1) * T)
        in_maps.append({
            "xt_in": np.ascontiguousarray(xt_full[:, sl]),
            "W1": W1, "b1": b1, "W2": W2, "b2": b2, "Wg": Wg, "bg": bg,
            "ones_in": _ONES,
        })
    return in_maps


def kernel(text_features, technical_features, W1, b1, W2, b2, Wg, bg):
    global LAST_RESULTS
    nc = _get_compiled()
    in_maps = make_in_maps(dict(
        text_features=text_features, technical_features=technical_features,
        W1=W1, b1=b1, W2=W2, b2=b2, Wg=Wg, bg=bg))
    last_exc = None
    for attempt in range(3):
        try:
            LAST_RESULTS = run_bass_kernel_spmd(nc, in_maps, core_ids=list(range(NCORES)))
            break
        except Exception as e:  # transient device/transfer errors: retry
            last_exc = e
            import time
            time.sleep(2.0 * (attempt + 1))
    else:
        raise last_exc
    return np.concatenate(
        [LAST_RESULTS.results[i]["out"] for i in range(NCORES)], axis=0)



# revision 10
# speedup vs baseline: 426.4004x; 1.0306x over previous
"""Trainium2 Bass kernel for nn_MixtureOfExperts (dense MoE, E=8 experts).

Data-parallel over 8 NeuronCores: each core processes B/8 = 2048 tokens with
all expert + gate weights replicated. No collectives.

Per-core algorithm (feature-major intermediates, fp32r matmuls):
  xT = concat(text, tech).T                     # [IN_D, T] host-side marshalling
  gates = softmax(x @ Wg + bg)                  # token-major [T, E]
  per expert e:
    hT_e = relu(W1[e].T @ xT + b1[e])           # feature-major [OUT_D, T]
    y_e  = hT_e.T @ W2[e]                       # token-major tiles, PSUM
    out += gates[:, e] * y_e                    # fused DVE scalar_tensor_tensor
  out += gates @ b2  (bias matmul, K=E)
"""
import os
import numpy as np
from contextlib import ExitStack

import concourse.bass as bass
import concourse.mybir as mybir
import concourse.tile as tile
from concourse import bacc
from concourse.bass_utils import run_bass_kernel_spmd

B, TEXT_D, TECH_D = 16384, 768, 256
IN_D, OUT_D, E = 1024, 1024, 8
NCORES = 8
T = B // NCORES          # 2048 tokens per core
C = 1024                 # token chunk per outer pass
NCH = T // C             # 2 chunks
P = 128
XT_BUFS = 1
KT = IN_D // P           # 8 k-tiles (also OUT_D // P)
FT = OUT_D // P          # 8 feature tiles
TT = C // P              # 8 token tiles per chunk
NC2 = C // 512           # 2 half-chunks of 512 tokens
F32 = mybir.dt.float32
F32R = mybir.dt.float32r
AF = mybir.ActivationFunctionType
ALU = mybir.AluOpType
AX = mybir.AxisListType


def build_kernel(nc: bass.Bass, reps: int = 1, *, ablate=()):
    """ablate (timing-only studies, wrong results):
    'dma'  = load W1/W2 once and reuse for every expert/chunk
    'gate' = constant gates, no gating matmuls/softmax/transposes
    'cons' = no PSUM consumers / DMA: pure PE instruction stream
    """
    from concourse.masks import make_identity

    xt_in = nc.dram_tensor("xt_in", [IN_D, T], F32R, kind="ExternalInput")
    W1 = nc.dram_tensor("W1", [E, IN_D, OUT_D], F32R, kind="ExternalInput")
    b1 = nc.dram_tensor("b1", [E, OUT_D], F32, kind="ExternalInput")
    W2 = nc.dram_tensor("W2", [E, OUT_D, OUT_D], F32R, kind="ExternalInput")
    b2 = nc.dram_tensor("b2", [E, OUT_D], F32R, kind="ExternalInput")
    Wg = nc.dram_tensor("Wg", [IN_D, E], F32R, kind="ExternalInput")
    bg = nc.dram_tensor("bg", [1, E], F32R, kind="ExternalInput")
    ones_in = nc.dram_tensor("ones_in", [1, P], F32R, kind="ExternalInput")
    out = nc.dram_tensor("out", [T, OUT_D], F32, kind="ExternalOutput")

    with tile.TileContext(nc) as tc, ExitStack() as ctx:
        consts = ctx.enter_context(tc.tile_pool(name="consts", bufs=1))
        xt_p = ctx.enter_context(tc.tile_pool(name="xt", bufs=2))
        h_p = ctx.enter_context(tc.tile_pool(name="h", bufs=1))
        oacc_p = ctx.enter_context(tc.tile_pool(name="oacc", bufs=1))
        w1_p = ctx.enter_context(tc.tile_pool(name="w1", bufs=4))
        w2_p = ctx.enter_context(tc.tile_pool(name="w2", bufs=1))
        gates_p = ctx.enter_context(tc.tile_pool(name="gates", bufs=1))
        gt_p = ctx.enter_context(tc.tile_pool(name="gt", bufs=2))
        sm_p = ctx.enter_context(tc.tile_pool(name="sm", bufs=2))
        # PSUM pools: xp 2 + lg 1 + ph 2 + y 3 = 8 banks
        pp_xp = ctx.enter_context(tc.tile_pool(name="pp_xp", bufs=1, space="PSUM"))
        pp_lg = ctx.enter_context(tc.tile_pool(name="pp_lg", bufs=1, space="PSUM"))
        pp_h = ctx.enter_context(tc.tile_pool(name="pp_h", bufs=3, space="PSUM"))
        pp_y = ctx.enter_context(tc.tile_pool(name="pp_y", bufs=3, space="PSUM"))

        identity = consts.tile([P, P], F32)
        make_identity(nc, identity[:])
        ones_r = consts.tile([1, P], F32R)
        nc.sync.dma_start(ones_r[:], ones_in[:, :])
        b1sb = consts.tile([P, E, FT], F32)
        nc.sync.dma_start(b1sb[:], b1.rearrange("e (ft p) -> p e ft", p=P))
        b2sb = consts.tile([E, OUT_D], F32R)
        nc.sync.dma_start(b2sb[:], b2[:, :])
        wg_sb = consts.tile([P, KT, E], F32R)
        nc.sync.dma_start(wg_sb[:], Wg.rearrange("(ko p) e -> p ko e", p=P))
        bg_sb = consts.tile([1, E], F32R)
        nc.sync.dma_start(bg_sb[:], bg[:, :])

        ab_dma = "dma" in ablate
        ab_gate = "gate" in ablate
        ab_cons = "cons" in ablate
        w1_sh = w2_sh = gates_const = h_sh = None
        if ab_dma:
            w2_sh = consts.tile([P, KT, OUT_D], F32R, name="w2_sh")
            for ko in range(KT):
                nc.sync.dma_start(w2_sh[:, ko], W2[0, ko * P:(ko + 1) * P, :])
            w1_sh = consts.tile([P, FT, KT, P], F32R, name="w1_sh")
            for f in range(FT):
                nc.sync.dma_start(
                    w1_sh[:, f],
                    W1[0, :, f * P:(f + 1) * P].rearrange("(ko p) f -> p ko f", p=P))
        if ab_gate:
            gates_const = consts.tile([P, TT, E], F32, name="gates_const")
            nc.vector.memset(gates_const[:], 0.125)
        if ab_cons:
            h_sh = consts.tile([P, FT, C], F32R, name="h_sh")
            nc.vector.memset(h_sh[:], 0.1)

        def load_w1(e, f):
            if ab_dma:
                return w1_sh[:, f]
            w1g = w1_p.tile([P, KT, P], F32R, tag="w1")
            nc.sync.dma_start(
                w1g[:], W1[e, :, f * P:(f + 1) * P].rearrange("(ko p) f -> p ko f", p=P))
            return w1g

        def layer1_block(e, f, w1g, h, xt):
            """One f-tile of layer 1: two interleaved 8-MM PSUM groups
            (alternating banks), then relu+bias to SBUF h."""
            ph0 = pp_h.tile([P, 512], F32, tag="ph")
            ph1 = pp_h.tile([P, 512], F32, tag="ph")
            for k in range(KT):
                nc.tensor.matmul(ph0[:], w1g[:, k], xt[:, k, 0:512],
                                 start=(k == 0), stop=(k == KT - 1))
                nc.tensor.matmul(ph1[:], w1g[:, k], xt[:, k, 512:1024],
                                 start=(k == 0), stop=(k == KT - 1))
            if not ab_cons:
                nc.scalar.activation(h[:, f, 0:512], ph0[:], AF.Relu,
                                     bias=b1sb[:, e, f:f + 1])
                nc.scalar.activation(h[:, f, 512:1024], ph1[:], AF.Relu,
                                     bias=b1sb[:, e, f:f + 1])

        def layer2_block(e, t, h, w2sb, gates, oacc):
            y0 = pp_y.tile([P, 512], F32, tag="y")
            y1 = pp_y.tile([P, 512], F32, tag="y")
            for k in range(KT):
                hk = h[:, k, t * P:(t + 1) * P]
                nc.tensor.matmul(y0[:], hk, w2sb[:, k, 0:512],
                                 start=(k == 0), stop=(k == KT - 1))
                nc.tensor.matmul(y1[:], hk, w2sb[:, k, 512:1024],
                                 start=(k == 0), stop=(k == KT - 1))
            if not ab_cons:
                g_e = gates[:, t, e:e + 1]
                nc.vector.scalar_tensor_tensor(
                    oacc[:, t, 0:512], y0[:], g_e, oacc[:, t, 0:512],
                    op0=ALU.mult, op1=ALU.add)
                nc.vector.scalar_tensor_tensor(
                    oacc[:, t, 512:1024], y1[:], g_e, oacc[:, t, 512:1024],
                    op0=ALU.mult, op1=ALU.add)

        def gate_finish(t, gates, oacc):
            """Transpose gates[:, t] -> gt, bias matmul gt @ b2 -> oacc init."""
            gtp = pp_xp.tile([P, P], F32, tag="xp")
            nc.tensor.transpose(gtp[:E, :], gates[:, t], identity[:])
            gt = gt_p.tile([E, P], F32R, tag="gt")
            nc.vector.tensor_copy(gt[:], gtp[:E, :])
            for c2 in range(2):
                yb = pp_y.tile([P, 512], F32, tag="y")
                nc.tensor.matmul(yb[:], gt[:], b2sb[:, c2 * 512:(c2 + 1) * 512],
                                 start=True, stop=True)
                if not ab_cons:
                    nc.vector.tensor_copy(
                        oacc[:, t, c2 * 512:(c2 + 1) * 512], yb[:])

        def load_xt(ch):
            t0 = (ch % NCH) * C
            xt = xt_p.tile([P, KT, C], F32R, tag="xt")
            for k in range(KT):
                nc.sync.dma_start(xt[:, k], xt_in[k * P:(k + 1) * P, t0:t0 + C])
            return xt

        NCHUNK = NCH * reps
        xt = load_xt(0)
        for ch in range(NCHUNK):
            t0 = (ch % NCH) * C
            gates = gates_p.tile([P, TT, E], F32, tag="gates")
            oacc = oacc_p.tile([P, TT, OUT_D], F32, tag="oacc")

            # ---- gating logit chains (tiny PE work; overlaps W1(e0) DMA
            # prefetch at the chunk boundary; softmax on DVE/ACT overlaps
            # the expert-0 layer-1 blocks that follow) ----
            if ab_gate:
                gates = gates_const
                if not ab_cons:
                    nc.vector.memset(oacc[:], 0.0)
            else:
                for t in range(TT):
                    lg = pp_lg.tile([P, E], F32, tag="lg")
                    nc.tensor.matmul(lg[:], ones_r[:], bg_sb[:], start=True, stop=False)
                    for k in range(KT):
                        nc.tensor.matmul(lg[:], xt[:, k, t * P:(t + 1) * P], wg_sb[:, k],
                                         start=False, stop=(k == KT - 1))
                    nmx = sm_p.tile([P, 1], F32, tag="nmx")
                    nc.vector.reduce_max(nmx[:], lg[:], axis=AX.X)
                    nc.vector.tensor_scalar_mul(nmx[:], nmx[:], -1.0)
                    g_t = gates[:, t]
                    nc.scalar.activation(g_t, lg[:], AF.Exp, bias=nmx[:])
                    sm = sm_p.tile([P, 1], F32, tag="sm")
                    nc.vector.reduce_sum(sm[:], g_t, axis=AX.X)
                    nc.vector.reciprocal(sm[:], sm[:])
                    nc.vector.tensor_scalar_mul(g_t, g_t, sm[:])

            # ---- expert loop; e=0 interleaves gate transpose/bias init
            # with its layer 2 ----
            for e in range(E):
                if ab_dma:
                    w2sb = w2_sh
                else:
                    w2sb = w2_p.tile([P, KT, OUT_D], F32R, tag="w2")
                    for ko in range(KT):
                        nc.scalar.dma_start(w2sb[:, ko], W2[e, ko * P:(ko + 1) * P, :])
                h = h_sh if ab_cons else h_p.tile([P, FT, C], F32R, tag="h")
                for f in range(FT):
                    layer1_block(e, f, load_w1(e, f), h, xt)
                if e == E - 1 and ch + 1 < NCHUNK:
                    # last xt reader done: start next chunk's xt transfer now
                    xt_next = load_xt(ch + 1)
                for t in range(TT):
                    if e == 0 and not ab_gate:
                        gate_finish(t, gates, oacc)
                    layer2_block(e, t, h, w2sb, gates, oacc)

            # ---- write out chunk ----
            if not ab_cons:
                for t in range(TT):
                    nc.sync.dma_start(out[t0 + t * P:t0 + (t + 1) * P, :], oacc[:, t])
            else:
                nc.sync.dma_start(out[t0:t0 + P, 0:4], b1sb[0:P, 0, 0:4])
            if ch + 1 < NCHUNK:
                xt = xt_next
    return nc


_ONES = np.ones((1, P), dtype=np.float32)

_compiled = {}


def _get_compiled(reps: int = 1):
    if reps not in _compiled:
        nc = bacc.Bacc(None, target_bir_lowering=False)
        build_kernel(nc, reps)
        nc.finalize()
        _compiled[reps] = nc
    return _compiled[reps]


LAST_RESULTS = None


def make_in_maps(inputs):
    text_features = np.asarray(inputs["text_features"], dtype=np.float32)
    technical_features = np.asarray(inputs["technical_features"], dtype=np.float32)
    xt_full = np.ascontiguousarray(
        np.concatenate([text_features, technical_features], axis=1).T)
    W1 = np.ascontiguousarray(np.asarray(inputs["W1"], dtype=np.float32))
    b1 = np.ascontiguousarray(np.asarray(inputs["b1"], dtype=np.float32))
    W2 = np.ascontiguousarray(np.asarray(inputs["W2"], dtype=np.float32))
    b2 = np.ascontiguousarray(np.asarray(inputs["b2"], dtype=np.float32))
    Wg = np.ascontiguousarray(np.asarray(inputs["Wg"], dtype=np.float32))
    bg = np.ascontiguousarray(
        np.asarray(inputs["bg"], dtype=np.float32).reshape(1, E))

    in_maps = []
    for i in range(NCORES):
        sl = slice(i * T, (i + 1) * T)
        in_maps.append({
            "xt_in": np.ascontiguousarray(xt_full[:, sl]),
            "W1": W1, "b1": b1, "W2": W2, "b2": b2, "Wg": Wg, "bg": bg,
            "ones_in": _ONES,
        })
    return in_maps


def kernel(text_features, technical_features, W1, b1, W2, b2, Wg, bg):
    global LAST_RESULTS
    nc = _get_compiled()
    in_maps = make_in_maps(dict(
        text_features=text_features, technical_features=technical_features,
        W1=W1, b1=b1, W2=W2, b2=b2, Wg=Wg, bg=bg))
    last_exc = None
    for attempt in range(3):
        try:
            LAST_RESULTS = run_bass_kernel_spmd(nc, in_maps, core_ids=list(range(NCORES)))
            break
        except Exception as e:  # transient device/transfer errors: retry
            last_exc = e
            import time
            time.sleep(2.0 * (attempt + 1))
    else:
        raise last_exc
    return np.concatenate(
        [LAST_RESULTS.results[i]["out"] for i in range(NCORES)], axis=0)

